# revision 20
# baseline (speedup 1.0000x reference)
"""Distributed RoPE multi-head attention for one TRN2 chip (8 NeuronCores).

Reference op (B=2, N=2048, C=1024, H=16, D=64, fp32):
    qkv = x @ w_qkv.T + b_qkv ; rope(q), rope(k)
    attn = softmax(q k^T / sqrt(D)) ; out = (attn v) @ w_proj.T + b_proj

Sharding: tensor-parallel over heads. Core c owns the head pair (2c, 2c+1)
for both batches: it computes its QKV slice, RoPE, and attention in
transposed layouts (feature on partitions, token on the free dim).

Key scheduling points vs the naive version:
  - The two heads' score matmuls (contraction D=64) are emitted adjacently
    at partition bases 0/64, so they run CONCURRENTLY on the two 64-row PE
    tiles (auto tile_position (0,0)/(64,0)).
  - Both heads' scores for one key tile share a [128,1024] PSUM group, so
    softmax exp is one ACT instruction per key tile (half the ACT overhead).
  - attn@v keeps a ones-column, so the softmax denominator rides along in
    row 64; the normalization happens AFTER the AllToAll with one batched
    [16,512] reciprocal + tiny selector matmuls (instead of 16 super-slow
    single-partition reciprocals + gpsimd broadcasts).
  - One combined AllToAll carries both heads + denominators (1.06 MB).
  - The output projection runs full-K (128) after the a2a.
  - Score groups for batch 0 are interleaved into the QKV phase so the
    ACT engine (the overall bottleneck) starts early and never starves.
"""

import os
import sys

import numpy as np

sys.path.insert(0, "/opt/trn_rl_repo")

import ml_dtypes  # noqa: E402

BF_NP = ml_dtypes.bfloat16

import concourse.bacc as bacc  # noqa: E402
import concourse.mybir as mybir  # noqa: E402
import concourse.tile as tile  # noqa: E402

B, N, C, H, D = 2, 2048, 1024, 16, 64
T = B * N                  # 4096 flattened tokens (batch-major)
NCORES = 8
TS = T // NCORES           # 512-token output slice per core
SCALE = float(D) ** -0.5
KK = C // 128              # 8 contraction tiles for the qkv matmul
KT = N // 128              # 16 key tiles per batch
QC = N // 512              # 4 query chunks per batch
VW = 130                   # vsb stride per key tile: (64 v + 1 ones) x 2 heads
EX_BUFS = 30

FP = mybir.dt.float32
BF = mybir.dt.bfloat16
AF = mybir.ActivationFunctionType
ALU = mybir.AluOpType

# q/k head-dim rows are stored interleaved as (d, d+32) pairs so that
# rotate_half becomes a pair swap WITHIN each 32-partition group, which
# the DVE stream_shuffle can do in one op (no partition-moving DMAs).
# ROT_PERM[r] = original d for stored row r; dot products are invariant.
ROT_PERM = [(r // 2) + 32 * (r % 2) for r in range(64)]
ROT_MASK = [i ^ 1 for i in range(32)]


def _build():
    from contextlib import ExitStack

    nc = bacc.Bacc(
        "TRN2",
        target_bir_lowering=False,
        debug=False,
        enable_asserts=False,
        num_devices=NCORES,
    )

    xT = nc.dram_tensor("xT", [C, T], BF, kind="ExternalInput").ap()
    wqkvT = nc.dram_tensor("wqkvT", [C, 384], BF, kind="ExternalInput").ap()
    bqkv = nc.dram_tensor("bqkv", [128, 3], FP, kind="ExternalInput").ap()
    cos2 = nc.dram_tensor("cos2", [128, T], BF, kind="ExternalInput").ap()
    sin2 = nc.dram_tensor("sin2", [128, T], BF, kind="ExternalInput").ap()
    wpT = nc.dram_tensor("wpT", [C, C], BF, kind="ExternalInput").ap()
    bproj = nc.dram_tensor("bproj", [128, 8], FP, kind="ExternalInput").ap()
    eye = nc.dram_tensor("eye", [128, 128], BF, kind="ExternalInput").ap()
    ones = nc.dram_tensor("ones", [128, 2 * B * KT], BF, kind="ExternalInput").ap()
    esel = nc.dram_tensor("esel", [16, 8 * 128], BF, kind="ExternalInput").ap()
    outT = nc.dram_tensor("outT", [C, TS], FP, kind="ExternalOutput").ap()

    with tile.TileContext(nc) as tc, ExitStack() as outer:
        pp = outer.enter_context(tc.tile_pool(name="persist", bufs=1))
        dp = outer.enter_context(tc.tile_pool(name="dram", bufs=1, space="DRAM"))
        es2 = ExitStack()
        # PSUM: score groups 2x[128,1024] = 4 banks (qkv+attention phases)
        psS = es2.enter_context(tc.tile_pool(name="ps_sg", bufs=2, space="PSUM"))
        asb = outer.enter_context(tc.tile_pool(name="exp", bufs=EX_BUFS))
        nsb = outer.enter_context(tc.tile_pool(name="small", bufs=1))

        qT = pp.tile([128, T], BF, name="qT")
        kT = pp.tile([128, T], BF, name="kT")
        vT = pp.tile([128, T], BF, name="vT")
        vsb = pp.tile([128, B * KT * VW], BF, name="vsb")
        eye_sb = pp.tile([128, 128], BF, name="eye_sb")
        bq_sb = pp.tile([128, 3], FP, name="bq_sb")
        bp_sb = pp.tile([128, 8], FP, name="bp_sb")
        esel_sb = pp.tile([16, 8 * 128], BF, name="esel_sb")

        a2a_in = dp.tile([NCORES, 130, 512], BF, name="a2a_in")
        a2a_out = dp.tile([NCORES, 130, 512], BF, name="a2a_out")

        # ---- input loads (scalar/ACT queue stays clean for exp) ----
        nc.sync.dma_start(eye_sb[:], eye)
        nc.sync.dma_start(bq_sb[:], bqkv)
        nc.gpsimd.dma_start(bp_sb[:], bproj)
        nc.gpsimd.dma_start(esel_sb[:], esel)

        es = ExitStack()
        xs = es.enter_context(tc.tile_pool(name="xs", bufs=1))
        # qkv chains (2 banks) + v transposes (1 bank)
        psA = es.enter_context(tc.tile_pool(name="ps_qkv", bufs=1, space="PSUM"))

        wq = xs.tile([128, KK * 384], BF, name="wq")
        for kk in range(KK):
            nc.sync.dma_start(
                wq[:, kk * 384 : (kk + 1) * 384], wqkvT[kk * 128 : (kk + 1) * 128, :]
            )
        cos_sb = xs.tile([128, T], BF, name="cos_sb")
        sin_sb = xs.tile([128, T], BF, name="sin_sb")
        nc.gpsimd.dma_start(cos_sb[:], cos2)
        nc.gpsimd.dma_start(sin_sb[:], sin2)

        xfull = [xs.tile([128, T], BF, name="xf", tag=f"xf{kk}") for kk in range(KK)]
        for half in range(2):
            hs = slice(half * (T // 2), (half + 1) * (T // 2))
            for kk in range(KK):
                (nc.sync, nc.gpsimd)[kk % 2].dma_start(
                    xfull[kk][:, hs], xT[kk * 128 : (kk + 1) * 128, hs]
                )

        # ones columns of vsb (col 64 of every 65-wide half-block)
        ov = vsb[:].rearrange("p (t c) -> p t c", c=65)
        nc.sync.dma_start(ov[:, :, 64:65], ones.rearrange("p (f o) -> p f o", o=1))

        # ---- score emission (heads concurrent on the 64-row PE tiles) ----
        ex_map = {}

        def emit_sg(b, qc, kt):
            """Scores + exp for one key tile, both heads."""
            sg = psS.tile([128, 1024], FP, name="sg", tag="sg")
            qs = slice(b * N + qc * 512, b * N + (qc + 1) * 512)
            ks = slice(b * N + kt * 128, b * N + (kt + 1) * 128)
            nc.tensor.matmul(
                sg[:, 0:512], lhsT=kT[0:64, ks], rhs=qT[0:64, qs],
                start=True, stop=True,
            )
            nc.tensor.matmul(
                sg[:, 512:1024], lhsT=kT[64:128, ks], rhs=qT[64:128, qs],
                start=True, stop=True,
            )
            ex = asb.tile([128, 1024], BF, name="ex", tag="ex")
            nc.scalar.activation(ex[:], sg[:], AF.Exp, scale=SCALE)
            ex_map[(b, qc, kt)] = ex

        # ---- qkv projection + rope + v retile for one 512-token chunk ----
        def emit_qkv_chunk(t):
            sl = slice(t * 512, (t + 1) * 512)
            dsts = (qT, kT, vT)
            for m in range(3):
                ps = psA.tile([128, 512], FP, name="psqkv", tag="qkv", bufs=2)
                for kk in range(KK):
                    col = kk * 384 + m * 128
                    nc.tensor.matmul(
                        ps[:],
                        lhsT=wq[:, col : col + 128],
                        rhs=xfull[kk][:, sl],
                        start=(kk == 0),
                        stop=(kk == KK - 1),
                    )
                nc.vector.tensor_scalar_add(
                    dsts[m][:, sl], ps[:], bq_sb[:, m : m + 1]
                )
            for tgt in (qT, kT):
                t1 = nsb.tile([128, 512], BF, name="rope1", tag="rope1", bufs=3)
                t2 = nsb.tile([128, 512], BF, name="rope2", tag="rope2", bufs=3)
                t3 = nsb.tile([128, 512], BF, name="rope3", tag="rope3", bufs=3)
                nc.vector.tensor_mul(t1[:], tgt[:, sl], cos_sb[:, sl])
                nc.vector.stream_shuffle(t2[:], tgt[:, sl], ROT_MASK)
                nc.vector.tensor_mul(t3[:], t2[:], sin_sb[:, sl])
                nc.vector.tensor_add(tgt[:, sl], t1[:], t3[:])
            for tj in range(4):
                ti = t * 4 + tj
                pt = psA.tile([128, 128], BF, name="ptr", tag="tr", bufs=1)
                nc.tensor.transpose(
                    pt[:], vT[:, ti * 128 : (ti + 1) * 128], eye_sb[:]
                )
                base = ti * VW
                nc.vector.tensor_copy(vsb[:, base : base + 64], pt[:, 0:64])
                nc.vector.tensor_copy(vsb[:, base + 65 : base + 129], pt[:, 64:128])

        # ---- interleaved qkv + early scores (batch 0) ----
        EARLY = {
            1: [(0, 0, kt) for kt in range(4)],
            2: [(0, 0, kt) for kt in range(4, 8)] + [(0, 1, kt) for kt in range(4)],
            3: [(0, 1, kt) for kt in range(4, 8)] + [(0, 2, kt) for kt in range(4)],
            4: [(0, 2, kt) for kt in range(4, 8)],
            5: [(0, 3, kt) for kt in range(2)],
        }
        for t in range(8):
            emit_qkv_chunk(t)
            for key in EARLY.get(t, []):
                emit_sg(*key)
        es.close()  # release x/wq/cos tiles + qkv/transpose psum banks

        # proj-phase SBUF (reuses the x/wq space) + attn@v accumulators
        p3 = outer.enter_context(tc.tile_pool(name="p3", bufs=1))
        psP = es2.enter_context(tc.tile_pool(name="ps_po", bufs=1, space="PSUM"))

        wp = p3.tile([128, 8 * C], BF, name="wp")
        for j in range(8):
            nc.gpsimd.dma_start(
                wp[:, j * C : (j + 1) * C], wpT[j * 128 : (j + 1) * 128, :]
            )

        def emit_block(b, qc):
            """attn@v + denominator for one (batch, query-chunk) block."""
            pending = [kt for kt in range(KT) if (b, qc, kt) not in ex_map]
            for i in range(min(2, len(pending))):
                emit_sg(b, qc, pending[i])
            pi = 2
            po = [
                psP.tile([65, 512], FP, name=f"po{e}", tag=f"po{e}", bufs=1)
                for e in range(2)
            ]
            for kt in range(KT):
                ex = ex_map[(b, qc, kt)]
                vcol = (b * KT + kt) * VW
                for e in range(2):
                    nc.tensor.matmul(
                        po[e][:],
                        lhsT=vsb[:, vcol + e * 65 : vcol + e * 65 + 65],
                        rhs=ex[:, e * 512 : (e + 1) * 512],
                        start=(kt == 0),
                        stop=(kt == KT - 1),
                    )
                del ex_map[(b, qc, kt)]
                if pi < len(pending):
                    emit_sg(b, qc, pending[pi])
                    pi += 1
            j = b * QC + qc
            for e in range(2):
                an = nsb.tile([65, 512], BF, name="an", tag="an", bufs=4)
                nc.vector.tensor_copy(an[:], po[e][:])
                nc.sync.dma_start(a2a_in[j, e * 64 : (e + 1) * 64, :], an[0:64, :])
                nc.sync.dma_start(a2a_in[j, 128 + e : 129 + e, :], an[64:65, :])

        for b in range(B):
            for qc in range(QC):
                emit_block(b, qc)

        nc.gpsimd.collective_compute(
            "AllToAll",
            ALU.bypass,
            replica_groups=[list(range(NCORES))],
            ins=[a2a_in.opt()],
            outs=[a2a_out.opt()],
        )

        # ---- post-a2a: batched softmax normalize + output projection ----
        es2.close()  # release score-group + po psum banks
        psY = outer.enter_context(tc.tile_pool(name="ps_y", bufs=1, space="PSUM"))

        ga = []
        for m in range(8):
            g = p3.tile([128, 512], BF, name="ga", tag=f"ga{m}")
            nc.sync.dma_start(g[:], a2a_out[m, 0:128, :])
            ga.append(g)
        denq = p3.tile([16, 512], BF, name="denq")
        for j in range(8):
            nc.gpsimd.dma_start(denq[2 * j : 2 * j + 2, :], a2a_out[j, 128:130, :])
        rcp = p3.tile([16, 512], BF, name="rcp")
        with nc.allow_low_precision(reason="bf16 softmax denom reciprocal"):
            nc.vector.reciprocal(rcp[:], denq[:])

        # j-pipelined: normalize block j, then immediately stream it into the
        # first-half projection chains; second half re-streams gan.
        gan = []
        pys = [psY.tile([128, 512], FP, name=f"pya{m}", tag=f"pya{m}") for m in range(4)]
        for j in range(8):
            dv = psY.tile([128, 512], FP, name="dv", tag="dv", bufs=2)
            nc.tensor.matmul(
                dv[:], lhsT=esel_sb[:, j * 128 : (j + 1) * 128], rhs=rcp[:],
                start=True, stop=True,
            )
            g2 = p3.tile([128, 512], BF, name="gan", tag=f"gan{j}")
            nc.vector.tensor_mul(g2[:], ga[j][:], dv[:])
            gan.append(g2)
            for m in range(4):
                nc.tensor.matmul(
                    pys[m][:],
                    lhsT=wp[:, j * C + m * 128 : j * C + (m + 1) * 128],
                    rhs=g2[:],
                    start=(j == 0),
                    stop=(j == 7),
                )
        for m in range(4):
            ysb = p3.tile([128, 512], FP, name="ysb", tag="ysb", bufs=4)
            nc.vector.tensor_scalar_add(ysb[:], pys[m][:], bp_sb[:, m : m + 1])
            (nc.sync, nc.gpsimd)[m % 2].dma_start(
                outT[m * 128 : (m + 1) * 128, :], ysb[:]
            )
        for m in range(4, 8):
            py = psY.tile([128, 512], FP, name="py2", tag="py2", bufs=2)
            for j in range(8):
                nc.tensor.matmul(
                    py[:],
                    lhsT=wp[:, j * C + m * 128 : j * C + (m + 1) * 128],
                    rhs=gan[j][:],
                    start=(j == 0),
                    stop=(j == 7),
                )
            ysb = p3.tile([128, 512], FP, name="ysb", tag="ysb", bufs=4)
            nc.vector.tensor_scalar_add(ysb[:], py[:], bp_sb[:, m : m + 1])
            (nc.sync, nc.gpsimd)[m % 2].dma_start(
                outT[m * 128 : (m + 1) * 128, :], ysb[:]
            )

    nc.compile()
    return nc


def _prep_inputs(inputs):
    """Full inputs -> per-core in_maps (host-side reshapes only)."""
    x = np.asarray(inputs["x"], dtype=np.float32)
    cos = np.asarray(inputs["cos"], dtype=np.float32)
    sin = np.asarray(inputs["sin"], dtype=np.float32)
    w_qkv = np.asarray(inputs["w_qkv"], dtype=np.float32)
    b_qkv = np.asarray(inputs["b_qkv"], dtype=np.float32)
    w_proj = np.asarray(inputs["w_proj"], dtype=np.float32)
    b_proj = np.asarray(inputs["b_proj"], dtype=np.float32)

    xTf = np.ascontiguousarray(x.reshape(T, C).T).astype(BF_NP)
    perm = np.array(ROT_PERM)
    cosT = cos[0, 0].T  # [64, 2048]
    sinT = sin[0, 0].T.copy()
    sinT[: D // 2] *= -1.0  # fold rotate_half's sign into sin
    cos2 = np.ascontiguousarray(np.tile(cosT[perm], (2, B))).astype(BF_NP)
    sin2 = np.ascontiguousarray(np.tile(sinT[perm], (2, B))).astype(BF_NP)
    wpT = np.ascontiguousarray(w_proj.T).astype(BF_NP)
    bp = np.ascontiguousarray(b_proj.reshape(8, 128).T)
    eye = np.eye(128, dtype=BF_NP)
    ones = np.ones((128, 2 * B * KT), dtype=BF_NP)
    esel = np.zeros((16, 8 * 128), dtype=BF_NP)
    for m in range(8):
        esel[2 * m, m * 128 : m * 128 + 64] = 1.0
        esel[2 * m + 1, m * 128 + 64 : m * 128 + 128] = 1.0

    in_maps = []
    for c in range(NCORES):
        rows = []
        bq = np.zeros((128, 3), dtype=np.float32)
        for m in range(3):
            for e in range(2):
                g = 2 * c + e
                base = m * C + g * 64
                order = perm if m < 2 else np.arange(64)  # q/k rows pair-interleaved
                rows.extend((base + order).tolist())
                bq[e * 64 : (e + 1) * 64, m] = b_qkv[base + order]
        wqh = np.ascontiguousarray(w_qkv[rows].T).astype(BF_NP)  # [1024, 384]
        in_maps.append(
            {
                "xT": xTf,
                "wqkvT": wqh,
                "bqkv": np.ascontiguousarray(bq),
                "cos2": cos2,
                "sin2": sin2,
                "wpT": wpT,
                "bproj": bp,
                "eye": eye,
                "ones": ones,
                "esel": esel,
            }
        )
    return in_maps


_NC_CACHE = None
last_results = None


def _install_ntff_hook():
    """Best-effort: register the axon NTFF profiling hook that the boot
    skipped (the image's antenv lacks axon_hooks). Trace-mode only."""
    try:
        import types

        if "antenv.axon_hooks" not in sys.modules:
            mod = types.ModuleType("antenv.axon_hooks")
            mod._hook = None
            mod.set_axon_ntff_profile_hook = lambda h: setattr(mod, "_hook", h)
            mod.get_axon_ntff_profile_hook = lambda: mod._hook
            sys.modules["antenv.axon_hooks"] = mod
            import antenv

            antenv.axon_hooks = mod
        import antenv.axon_hooks as ah

        if ah.get_axon_ntff_profile_hook() is None:
            if "/root/.axon_site" not in sys.path:
                sys.path.insert(0, "/root/.axon_site")
            from trn_agent_boot.trn_boot import _ntff_profile_via_ctypes

            hook = _ntff_profile_via_ctypes("/opt/axon/libaxon_pjrt.so")
            if hook is not None:
                ah.set_axon_ntff_profile_hook(hook)
        # artifact upload needs a bucket this sandbox doesn't have
        import concourse.bass_utils as bu

        bu.upload_artifacts = lambda tmpdir: tmpdir
    except Exception as e:  # pragma: no cover - profiling is optional
        print(f"ntff hook install failed: {e}", file=sys.stderr)


def kernel(**inputs):
    global _NC_CACHE, last_results
    from concourse.bass_utils import run_bass_kernel_spmd

    if _NC_CACHE is None:
        _NC_CACHE = _build()
    in_maps = _prep_inputs(inputs)
    trace = os.environ.get("KBENCH_TRACE", "0") == "1"
    if trace:
        _install_ntff_hook()
    res = None
    for attempt in range(3):
        try:
            res = run_bass_kernel_spmd(
                _NC_CACHE, in_maps, core_ids=list(range(NCORES)), trace=trace
            )
            break
        except Exception:
            if attempt == 2:
                raise
            import time as _time

            _time.sleep(20)
    last_results = res
    shards = [res.results[c]["outT"].T for c in range(NCORES)]  # each [512, 1024]
    y = np.concatenate(shards, axis=0).reshape(B, N, C)
    return np.ascontiguousarray(y.astype(np.float32))


# revision 24
# speedup vs baseline: 1.1913x; 1.1913x over previous
"""Distributed RoPE multi-head attention for one TRN2 chip (8 NeuronCores).

Reference op (B=2, N=2048, C=1024, H=16, D=64, fp32):
    qkv = x @ w_qkv.T + b_qkv ; rope(q), rope(k)
    attn = softmax(q k^T / sqrt(D)) ; out = (attn v) @ w_proj.T + b_proj

Sharding: tensor-parallel over heads. Core c owns the head pair (2c, 2c+1)
for both batches: it computes its QKV slice, RoPE, and attention in
transposed layouts (feature on partitions, token on the free dim).

Key scheduling points vs the naive version:
  - The two heads' score matmuls (contraction D=64) are emitted adjacently
    at partition bases 0/64, so they run CONCURRENTLY on the two 64-row PE
    tiles (auto tile_position (0,0)/(64,0)).
  - Both heads' scores for one key tile share a [128,1024] PSUM group, so
    softmax exp is one ACT instruction per key tile (half the ACT overhead).
  - attn@v keeps a ones-column, so the softmax denominator rides along in
    row 64; the normalization happens AFTER the AllToAll with one batched
    [16,512] reciprocal + tiny selector matmuls (instead of 16 super-slow
    single-partition reciprocals + gpsimd broadcasts).
  - One combined AllToAll carries both heads + denominators (1.06 MB).
  - The output projection runs full-K (128) after the a2a.
  - Score groups for batch 0 are interleaved into the QKV phase so the
    ACT engine (the overall bottleneck) starts early and never starves.
"""

import os
import sys

import numpy as np

sys.path.insert(0, "/opt/trn_rl_repo")

import ml_dtypes  # noqa: E402

BF_NP = ml_dtypes.bfloat16

import concourse.bacc as bacc  # noqa: E402
import concourse.mybir as mybir  # noqa: E402
import concourse.tile as tile  # noqa: E402

B, N, C, H, D = 2, 2048, 1024, 16, 64
T = B * N                  # 4096 flattened tokens (batch-major)
NCORES = 8
TS = T // NCORES           # 512-token output slice per core
SCALE = float(D) ** -0.5
KK = C // 128              # 8 contraction tiles for the qkv matmul
KT = N // 128              # 16 key tiles per batch
QC = N // 512              # 4 query chunks per batch
VW = 130                   # vsb stride per key tile: (64 v + 1 ones) x 2 heads
EX_BUFS = 30

FP = mybir.dt.float32
BF = mybir.dt.bfloat16
AF = mybir.ActivationFunctionType
ALU = mybir.AluOpType

# q/k head-dim rows are stored interleaved as (d, d+32) pairs so that
# rotate_half becomes a pair swap WITHIN each 32-partition group, which
# the DVE stream_shuffle can do in one op (no partition-moving DMAs).
# ROT_PERM[r] = original d for stored row r; dot products are invariant.
ROT_PERM = [(r // 2) + 32 * (r % 2) for r in range(64)]
ROT_MASK = [i ^ 1 for i in range(32)]


def _build():
    from contextlib import ExitStack

    nc = bacc.Bacc(
        "TRN2",
        target_bir_lowering=False,
        debug=False,
        enable_asserts=False,
        num_devices=NCORES,
    )

    xT = nc.dram_tensor("xT", [C, T], BF, kind="ExternalInput").ap()
    wqkvT = nc.dram_tensor("wqkvT", [C, 384], BF, kind="ExternalInput").ap()
    bqkv = nc.dram_tensor("bqkv", [128, 3], FP, kind="ExternalInput").ap()
    cos2 = nc.dram_tensor("cos2", [128, T], BF, kind="ExternalInput").ap()
    sin2 = nc.dram_tensor("sin2", [128, T], BF, kind="ExternalInput").ap()
    wpT = nc.dram_tensor("wpT", [C, C], BF, kind="ExternalInput").ap()
    bproj = nc.dram_tensor("bproj", [128, 8], FP, kind="ExternalInput").ap()
    eye = nc.dram_tensor("eye", [128, 128], BF, kind="ExternalInput").ap()
    ones = nc.dram_tensor("ones", [128, 2 * B * KT], BF, kind="ExternalInput").ap()
    esel = nc.dram_tensor("esel", [16, 8 * 128], BF, kind="ExternalInput").ap()
    outT = nc.dram_tensor("outT", [C, TS], FP, kind="ExternalOutput").ap()

    with tile.TileContext(nc) as tc, ExitStack() as outer:
        pp = outer.enter_context(tc.tile_pool(name="persist", bufs=1))
        dp = outer.enter_context(tc.tile_pool(name="dram", bufs=1, space="DRAM"))
        es2 = ExitStack()
        # PSUM: score groups 2x[128,1024] = 4 banks (qkv+attention phases)
        psS = es2.enter_context(tc.tile_pool(name="ps_sg", bufs=2, space="PSUM"))
        asb = outer.enter_context(tc.tile_pool(name="exp", bufs=EX_BUFS))
        nsb = outer.enter_context(tc.tile_pool(name="small", bufs=1))

        qT = pp.tile([128, T], BF, name="qT")
        kT = pp.tile([128, T], BF, name="kT")
        vT = pp.tile([128, T], BF, name="vT")
        vsb = pp.tile([128, B * KT * VW], BF, name="vsb")
        eye_sb = pp.tile([128, 128], BF, name="eye_sb")
        bq_sb = pp.tile([128, 3], FP, name="bq_sb")
        bp_sb = pp.tile([128, 8], FP, name="bp_sb")
        esel_sb = pp.tile([16, 8 * 128], BF, name="esel_sb")

        a2a_in = dp.tile([NCORES, 130, 512], BF, name="a2a_in")
        a2a_out = dp.tile([NCORES, 130, 512], BF, name="a2a_out")

        # ---- input loads (scalar/ACT queue stays clean for exp) ----
        nc.sync.dma_start(eye_sb[:], eye)
        nc.sync.dma_start(bq_sb[:], bqkv)
        nc.gpsimd.dma_start(bp_sb[:], bproj)
        nc.gpsimd.dma_start(esel_sb[:], esel)

        es = ExitStack()
        xs = es.enter_context(tc.tile_pool(name="xs", bufs=1))
        # qkv chains (2 banks) + v transposes (1 bank)
        psA = es.enter_context(tc.tile_pool(name="ps_qkv", bufs=1, space="PSUM"))

        wq = xs.tile([128, KK * 384], BF, name="wq")
        for kk in range(KK):
            nc.sync.dma_start(
                wq[:, kk * 384 : (kk + 1) * 384], wqkvT[kk * 128 : (kk + 1) * 128, :]
            )
        cos_sb = xs.tile([128, T], BF, name="cos_sb")
        sin_sb = xs.tile([128, T], BF, name="sin_sb")
        nc.gpsimd.dma_start(cos_sb[:], cos2)
        nc.gpsimd.dma_start(sin_sb[:], sin2)

        # x per 512-token chunk, chunk-major so chunk 0 lands first
        xfull = [xs.tile([128, T], BF, name="xf", tag=f"xf{kk}") for kk in range(KK)]
        for t in range(8):
            for kk in range(KK):
                sl = slice(t * 512, (t + 1) * 512)
                (nc.sync, nc.gpsimd)[kk % 2].dma_start(
                    xfull[kk][:, sl], xT[kk * 128 : (kk + 1) * 128, sl]
                )

        # ones columns of vsb (col 64 of every 65-wide half-block)
        ov = vsb[:].rearrange("p (t c) -> p t c", c=65)
        nc.sync.dma_start(ov[:, :, 64:65], ones.rearrange("p (f o) -> p f o", o=1))

        # ---- score emission (heads concurrent on the 64-row PE tiles) ----
        ex_map = {}

        def emit_sg(b, qc, kt):
            """Scores + exp for one key tile, both heads."""
            sg = psS.tile([128, 1024], FP, name="sg", tag="sg")
            qs = slice(b * N + qc * 512, b * N + (qc + 1) * 512)
            ks = slice(b * N + kt * 128, b * N + (kt + 1) * 128)
            nc.tensor.matmul(
                sg[:, 0:512], lhsT=kT[0:64, ks], rhs=qT[0:64, qs],
                start=True, stop=True,
            )
            nc.tensor.matmul(
                sg[:, 512:1024], lhsT=kT[64:128, ks], rhs=qT[64:128, qs],
                start=True, stop=True,
            )
            ex = asb.tile([128, 1024], BF, name="ex", tag="ex")
            nc.scalar.activation(ex[:], sg[:], AF.Exp, scale=SCALE)
            ex_map[(b, qc, kt)] = ex

        # ---- qkv projection + rope + v retile for one 512-token chunk ----
        def emit_qkv_chunk(t):
            sl = slice(t * 512, (t + 1) * 512)
            dsts = (qT, kT, vT)
            for m in range(3):
                ps = psA.tile([128, 512], FP, name="psqkv", tag="qkv", bufs=2)
                for kk in range(KK):
                    col = kk * 384 + m * 128
                    nc.tensor.matmul(
                        ps[:],
                        lhsT=wq[:, col : col + 128],
                        rhs=xfull[kk][:, sl],
                        start=(kk == 0),
                        stop=(kk == KK - 1),
                    )
                nc.vector.tensor_scalar_add(
                    dsts[m][:, sl], ps[:], bq_sb[:, m : m + 1]
                )
            for tgt in (qT, kT):
                t1 = nsb.tile([128, 512], BF, name="rope1", tag="rope1", bufs=3)
                t2 = nsb.tile([128, 512], BF, name="rope2", tag="rope2", bufs=3)
                t3 = nsb.tile([128, 512], BF, name="rope3", tag="rope3", bufs=3)
                nc.vector.tensor_mul(t1[:], tgt[:, sl], cos_sb[:, sl])
                nc.vector.stream_shuffle(t2[:], tgt[:, sl], ROT_MASK)
                nc.vector.tensor_mul(t3[:], t2[:], sin_sb[:, sl])
                nc.vector.tensor_add(tgt[:, sl], t1[:], t3[:])
            for tj in range(4):
                ti = t * 4 + tj
                pt = psA.tile([128, 128], BF, name="ptr", tag="tr", bufs=1)
                nc.tensor.transpose(
                    pt[:], vT[:, ti * 128 : (ti + 1) * 128], eye_sb[:]
                )
                base = ti * VW
                nc.vector.tensor_copy(vsb[:, base : base + 64], pt[:, 0:64])
                nc.vector.tensor_copy(vsb[:, base + 65 : base + 129], pt[:, 64:128])

        # ---- global score-group stream in block consumption order ----
        # Emitted progressively: during qkv only keys/queries already rope'd
        # are eligible; during blocks the pump keeps ACT fed ~EX_BUFS ahead.
        sg_queue = [
            (b, qc, kt) for b in range(B) for qc in range(QC) for kt in range(KT)
        ]
        state = {"head": 0, "live": 0, "chunks": 0}

        def pump(n):
            while n > 0 and state["head"] < len(sg_queue):
                b, qc, kt = sg_queue[state["head"]]
                ready = (b * QC + qc) < state["chunks"] and (
                    b * KT + kt
                ) * 128 < state["chunks"] * 512
                if not ready or state["live"] >= EX_BUFS - 2:
                    return
                emit_sg(b, qc, kt)
                state["head"] += 1
                state["live"] += 1
                n -= 1

        for t in range(8):
            emit_qkv_chunk(t)
            state["chunks"] = t + 1
            pump(6)
        es.close()  # release x/wq/cos tiles + qkv/transpose psum banks

        # proj-phase SBUF (reuses the x/wq space) + attn@v accumulators
        p3 = outer.enter_context(tc.tile_pool(name="p3", bufs=1))
        psP = es2.enter_context(tc.tile_pool(name="ps_po", bufs=1, space="PSUM"))

        wp = p3.tile([128, 8 * C], BF, name="wp")
        for j in range(8):
            nc.gpsimd.dma_start(
                wp[:, j * C : (j + 1) * C], wpT[j * 128 : (j + 1) * 128, :]
            )

        def emit_block(b, qc):
            """attn@v + denominator for one (batch, query-chunk) block."""
            # this block's own score groups must all be emitted first
            while (b, qc, KT - 1) not in ex_map:
                pump(1)
            po = [
                psP.tile([65, 512], FP, name=f"po{e}", tag=f"po{e}", bufs=1)
                for e in range(2)
            ]
            for kt in range(KT):
                ex = ex_map[(b, qc, kt)]
                vcol = (b * KT + kt) * VW
                for e in range(2):
                    nc.tensor.matmul(
                        po[e][:],
                        lhsT=vsb[:, vcol + e * 65 : vcol + e * 65 + 65],
                        rhs=ex[:, e * 512 : (e + 1) * 512],
                        start=(kt == 0),
                        stop=(kt == KT - 1),
                    )
                del ex_map[(b, qc, kt)]
                state["live"] -= 1
                pump(2)
            j = b * QC + qc
            for e in range(2):
                an = nsb.tile([65, 512], BF, name="an", tag="an", bufs=4)
                nc.vector.tensor_copy(an[:], po[e][:])
                nc.sync.dma_start(a2a_in[j, e * 64 : (e + 1) * 64, :], an[0:64, :])
                nc.sync.dma_start(a2a_in[j, 128 + e : 129 + e, :], an[64:65, :])

        for b in range(B):
            for qc in range(QC):
                emit_block(b, qc)

        nc.gpsimd.collective_compute(
            "AllToAll",
            ALU.bypass,
            replica_groups=[list(range(NCORES))],
            ins=[a2a_in.opt()],
            outs=[a2a_out.opt()],
        )

        # ---- post-a2a: batched softmax normalize + output projection ----
        es2.close()  # release score-group + po psum banks
        psY = outer.enter_context(tc.tile_pool(name="ps_y", bufs=1, space="PSUM"))

        ga = []
        for m in range(8):
            g = p3.tile([128, 512], BF, name="ga", tag=f"ga{m}")
            nc.sync.dma_start(g[:], a2a_out[m, 0:128, :])
            ga.append(g)
        denq = p3.tile([16, 512], BF, name="denq")
        for j in range(8):
            nc.gpsimd.dma_start(denq[2 * j : 2 * j + 2, :], a2a_out[j, 128:130, :])
        rcp = p3.tile([16, 512], BF, name="rcp")
        with nc.allow_low_precision(reason="bf16 softmax denom reciprocal"):
            nc.vector.reciprocal(rcp[:], denq[:])

        # normalize blocks (divisor broadcast via tiny selector matmuls, all
        # in one PE mode), then stream them into the projection chains.
        gan = []
        pys = [psY.tile([128, 512], FP, name=f"pya{m}", tag=f"pya{m}") for m in range(4)]
        for j in range(8):
            dv = psY.tile([128, 512], FP, name="dv", tag="dv", bufs=2)
            nc.tensor.matmul(
                dv[:], lhsT=esel_sb[:, j * 128 : (j + 1) * 128], rhs=rcp[:],
                start=True, stop=True,
            )
            g2 = p3.tile([128, 512], BF, name="gan", tag=f"gan{j}")
            nc.vector.tensor_mul(g2[:], ga[j][:], dv[:])
            gan.append(g2)
        for j in range(8):
            for m in range(4):
                nc.tensor.matmul(
                    pys[m][:],
                    lhsT=wp[:, j * C + m * 128 : j * C + (m + 1) * 128],
                    rhs=gan[j][:],
                    start=(j == 0),
                    stop=(j == 7),
                )
        for m in range(4):
            ysb = p3.tile([128, 512], FP, name="ysb", tag="ysb", bufs=4)
            nc.vector.tensor_scalar_add(ysb[:], pys[m][:], bp_sb[:, m : m + 1])
            (nc.sync, nc.gpsimd)[m % 2].dma_start(
                outT[m * 128 : (m + 1) * 128, :], ysb[:]
            )
        for m in range(4, 8):
            py = psY.tile([128, 512], FP, name="py2", tag="py2", bufs=2)
            for j in range(8):
                nc.tensor.matmul(
                    py[:],
                    lhsT=wp[:, j * C + m * 128 : j * C + (m + 1) * 128],
                    rhs=gan[j][:],
                    start=(j == 0),
                    stop=(j == 7),
                )
            ysb = p3.tile([128, 512], FP, name="ysb", tag="ysb", bufs=4)
            nc.vector.tensor_scalar_add(ysb[:], py[:], bp_sb[:, m : m + 1])
            (nc.sync, nc.gpsimd)[m % 2].dma_start(
                outT[m * 128 : (m + 1) * 128, :], ysb[:]
            )

    nc.compile()
    return nc


def _prep_inputs(inputs):
    """Full inputs -> per-core in_maps (host-side reshapes only)."""
    x = np.asarray(inputs["x"], dtype=np.float32)
    cos = np.asarray(inputs["cos"], dtype=np.float32)
    sin = np.asarray(inputs["sin"], dtype=np.float32)
    w_qkv = np.asarray(inputs["w_qkv"], dtype=np.float32)
    b_qkv = np.asarray(inputs["b_qkv"], dtype=np.float32)
    w_proj = np.asarray(inputs["w_proj"], dtype=np.float32)
    b_proj = np.asarray(inputs["b_proj"], dtype=np.float32)

    xTf = np.ascontiguousarray(x.reshape(T, C).T).astype(BF_NP)
    perm = np.array(ROT_PERM)
    cosT = cos[0, 0].T  # [64, 2048]
    sinT = sin[0, 0].T.copy()
    sinT[: D // 2] *= -1.0  # fold rotate_half's sign into sin
    cos2 = np.ascontiguousarray(np.tile(cosT[perm], (2, B))).astype(BF_NP)
    sin2 = np.ascontiguousarray(np.tile(sinT[perm], (2, B))).astype(BF_NP)
    wpT = np.ascontiguousarray(w_proj.T).astype(BF_NP)
    bp = np.ascontiguousarray(b_proj.reshape(8, 128).T)
    eye = np.eye(128, dtype=BF_NP)
    ones = np.ones((128, 2 * B * KT), dtype=BF_NP)
    esel = np.zeros((16, 8 * 128), dtype=BF_NP)
    for m in range(8):
        esel[2 * m, m * 128 : m * 128 + 64] = 1.0
        esel[2 * m + 1, m * 128 + 64 : m * 128 + 128] = 1.0

    in_maps = []
    for c in range(NCORES):
        rows = []
        bq = np.zeros((128, 3), dtype=np.float32)
        for m in range(3):
            for e in range(2):
                g = 2 * c + e
                base = m * C + g * 64
                order = perm if m < 2 else np.arange(64)  # q/k rows pair-interleaved
                rows.extend((base + order).tolist())
                bq[e * 64 : (e + 1) * 64, m] = b_qkv[base + order]
        wqh = np.ascontiguousarray(w_qkv[rows].T).astype(BF_NP)  # [1024, 384]
        in_maps.append(
            {
                "xT": xTf,
                "wqkvT": wqh,
                "bqkv": np.ascontiguousarray(bq),
                "cos2": cos2,
                "sin2": sin2,
                "wpT": wpT,
                "bproj": bp,
                "eye": eye,
                "ones": ones,
                "esel": esel,
            }
        )
    return in_maps


_NC_CACHE = None
last_results = None


def _install_ntff_hook():
    """Best-effort: register the axon NTFF profiling hook that the boot
    skipped (the image's antenv lacks axon_hooks). Trace-mode only."""
    try:
        import types

        if "antenv.axon_hooks" not in sys.modules:
            mod = types.ModuleType("antenv.axon_hooks")
            mod._hook = None
            mod.set_axon_ntff_profile_hook = lambda h: setattr(mod, "_hook", h)
            mod.get_axon_ntff_profile_hook = lambda: mod._hook
            sys.modules["antenv.axon_hooks"] = mod
            import antenv

            antenv.axon_hooks = mod
        import antenv.axon_hooks as ah

        if ah.get_axon_ntff_profile_hook() is None:
            if "/root/.axon_site" not in sys.path:
                sys.path.insert(0, "/root/.axon_site")
            from trn_agent_boot.trn_boot import _ntff_profile_via_ctypes

            hook = _ntff_profile_via_ctypes("/opt/axon/libaxon_pjrt.so")
            if hook is not None:
                ah.set_axon_ntff_profile_hook(hook)
        # artifact upload needs a bucket this sandbox doesn't have
        import concourse.bass_utils as bu

        bu.upload_artifacts = lambda tmpdir: tmpdir
    except Exception as e:  # pragma: no cover - profiling is optional
        print(f"ntff hook install failed: {e}", file=sys.stderr)


def kernel(**inputs):
    global _NC_CACHE, last_results
    from concourse.bass_utils import run_bass_kernel_spmd

    if _NC_CACHE is None:
        _NC_CACHE = _build()
    in_maps = _prep_inputs(inputs)
    trace = os.environ.get("KBENCH_TRACE", "0") == "1"
    if trace:
        _install_ntff_hook()
    res = None
    for attempt in range(3):
        try:
            res = run_bass_kernel_spmd(
                _NC_CACHE, in_maps, core_ids=list(range(NCORES)), trace=trace
            )
            break
        except Exception:
            if attempt == 2:
                raise
            import time as _time

            _time.sleep(20)
    last_results = res
    shards = [res.results[c]["outT"].T for c in range(NCORES)]  # each [512, 1024]
    y = np.concatenate(shards, axis=0).reshape(B, N, C)
    return np.ascontiguousarray(y.astype(np.float32))
